# revision 18
# baseline (speedup 1.0000x reference)
"""Trainium2 Bass kernel for nn_AbsorbanceLookup (retrieval_knn).

Contract: kernel(**inputs) -> np.ndarray. Takes the FULL unsharded inputs
(keys as in reference.setup_inputs()), distributes across 8 NeuronCores
(pure data parallel on the batch dim), returns the FULL [B] output.

Per-core layout (Bc = 32768 queries = [128, 256]):
  natural  : nat[p, f]        = query q = 256*p + f     (contiguous DMA)
  H        : H[i, 128*a + p]  = query q = 256*p + 128*a + i   (PE transpose of nat)
  T (psum) : T[i, 2*p + a]    = same queries, free dims permuted (MLP order)

MLP runs in feature-major orientation with bf16 matmuls (fp32 psum accum):
  L1: 64 zero-padded [32,128] lhsT blocks at quadrant tile positions
  L2/L3: [128,128] lhsT blocks, L3 accumulates two K-halves
  L4: lhsT = h3 column-chunk (M=batch), rhs = w4 -> psum columns
Exact-match path: nearest-grid indices via the 2^23 magic-round trick
(bit-exact with the reference's fp32 mul/add/compare sequence), flat-table
gpsimd ap_gather from a host-prenormalized (A-mean)/std table replicated
across partitions, then 16 strided copy_predicated merges.
"""

import sys

if "/opt/trn_rl_repo" not in sys.path:
    sys.path.insert(0, "/opt/trn_rl_repo")

import numpy as np
import ml_dtypes

import concourse.bass as bass
import concourse.tile as tile
from concourse import bacc, mybir
from concourse.bass_utils import run_bass_kernel_spmd

F32 = mybir.dt.float32
BF16 = mybir.dt.bfloat16
I16 = mybir.dt.int16
ALU = mybir.AluOpType
ACTF = mybir.ActivationFunctionType

B = 262144
N_CORES = 8
BC = B // N_CORES          # 32768 per core
P = 128
FC = BC // P               # 256 free columns in natural layout
N_CONCS = 16
N_WL = 601
TBL = N_CONCS * N_WL       # 9616
MAGIC = 8388608.0          # 2^23: x + MAGIC - MAGIC == round-to-nearest-int(x)
C_MEAN, C_STD = 30.0, 30.0
WL_MEAN, WL_STD = 500.0, 300.0
N_U = 64                   # query tiles of 512 for the MLP
UW = 512                   # queries per MLP tile


def build_nc(debug_outputs=False):
    nc = bacc.Bacc("TRN2", target_bir_lowering=False, debug=False,
                   num_devices=N_CORES)

    # ---- dram I/O ----
    d_cn = nc.dram_tensor("cn", [P, FC], F32, kind="ExternalInput").ap()
    d_wn = nc.dram_tensor("wn", [P, FC], F32, kind="ExternalInput").ap()
    d_xin = nc.dram_tensor("xin", [P, UW], BF16, kind="ExternalInput").ap()
    d_w1all = nc.dram_tensor("w1all", [P, 4096], BF16, kind="ExternalInput").ap()
    d_w2 = nc.dram_tensor("w2", [P, 256], BF16, kind="ExternalInput").ap()
    d_w3ab = nc.dram_tensor("w3ab", [P, 256], BF16, kind="ExternalInput").ap()
    d_w4 = nc.dram_tensor("w4", [P, 1], BF16, kind="ExternalInput").ap()
    d_b1 = nc.dram_tensor("b1", [P, 1], F32, kind="ExternalInput").ap()
    d_b2a = nc.dram_tensor("b2a", [P, 1], F32, kind="ExternalInput").ap()
    d_b2b = nc.dram_tensor("b2b", [P, 1], F32, kind="ExternalInput").ap()
    d_b3 = nc.dram_tensor("b3", [P, 1], F32, kind="ExternalInput").ap()
    d_b4 = nc.dram_tensor("b4r", [P, 1], F32, kind="ExternalInput").ap()
    d_tbl = nc.dram_tensor("tbl", [P, TBL], F32, kind="ExternalInput").ap()
    d_ident = nc.dram_tensor("ident", [P, P], F32, kind="ExternalInput").ap()
    d_sidx = nc.dram_tensor("sidx", [P, 2 * (BC // 8)], I16,
                            kind="ExternalInput").ap()
    d_out = nc.dram_tensor("out", [P, FC], F32, kind="ExternalOutput").ap()
    # debug outputs (raw tiles, H / T layouts)
    if debug_outputs:
        d_mask = nc.dram_tensor("maskH", [P, FC], mybir.dt.uint8,
                                kind="ExternalOutput").ap()
        d_gf = nc.dram_tensor("gfH", [P, FC], F32, kind="ExternalOutput").ap()
        d_ex = nc.dram_tensor("exactT", [P, FC], F32, kind="ExternalOutput").ap()
        d_it = nc.dram_tensor("interpT", [P, FC], F32, kind="ExternalOutput").ap()
        d_dst = nc.dram_tensor("dstraw", [P, BC // 8], F32,
                               kind="ExternalOutput").ap()

    with tile.TileContext(nc) as tc:
        with (
            tc.tile_pool(name="const", bufs=1) as cpool,
            tc.tile_pool(name="hit", bufs=1) as hpool,
            tc.tile_pool(name="mlp", bufs=3) as mpool,
            tc.tile_pool(name="ps_tr", bufs=2, space="PSUM") as ptr,
            tc.tile_pool(name="ps_l1", bufs=2, space="PSUM") as pl1,
            tc.tile_pool(name="ps_l2a", bufs=1, space="PSUM") as pl2a,
            tc.tile_pool(name="ps_l2b", bufs=1, space="PSUM") as pl2b,
            tc.tile_pool(name="ps_l3", bufs=1, space="PSUM") as pl3,
            tc.tile_pool(name="ps_l4", bufs=1, space="PSUM") as pl4,
        ):
            # ---- constants into SBUF ----
            def cin(ap_dram, shape, dtype, tag):
                t = cpool.tile(shape, dtype, tag=tag)
                nc.sync.dma_start(t[:], ap_dram)
                return t

            s_ident = cin(d_ident, [P, P], F32, "ident")
            s_cn = cin(d_cn, [P, FC], F32, "cn")
            s_wn = cin(d_wn, [P, FC], F32, "wn")
            s_xin = cin(d_xin, [P, UW], BF16, "xin")
            s_w1 = cin(d_w1all, [P, 4096], BF16, "w1all")
            s_w2 = cin(d_w2, [P, 256], BF16, "w2")
            s_w3 = cin(d_w3ab, [P, 256], BF16, "w3ab")
            s_w4 = cin(d_w4, [P, 1], BF16, "w4")
            s_b1 = cin(d_b1, [P, 1], F32, "b1")
            s_b2a = cin(d_b2a, [P, 1], F32, "b2a")
            s_b2b = cin(d_b2b, [P, 1], F32, "b2b")
            s_b3 = cin(d_b3, [P, 1], F32, "b3")
            s_b4 = cin(d_b4, [P, 1], F32, "b4r")
            s_tbl = cin(d_tbl, [P, TBL], F32, "tbl")
            s_sidx = cin(d_sidx, [P, 2 * (BC // 8)], I16, "sidx")

            # =========================================================
            # Hit path (H layout).  c/wl transposed into [128, 256]:
            # free index u' = 128*a + p  <->  query q = 256*p + 128*a + i
            # =========================================================
            cH = hpool.tile([P, FC], F32, tag="cH")
            wH = hpool.tile([P, FC], F32, tag="wH")
            for a in range(2):
                tp = ptr.tile([P, P], F32, tag="tr")
                nc.tensor.transpose(tp[:], s_cn[:, a * P:(a + 1) * P], s_ident[:])
                # cM = cT * 30 ; separate mul and add to match reference rounding
                nc.vector.tensor_scalar(cH[:, a * P:(a + 1) * P], tp[:],
                                        C_STD, None, ALU.mult)
                tp2 = ptr.tile([P, P], F32, tag="tr")
                nc.tensor.transpose(tp2[:], s_wn[:, a * P:(a + 1) * P], s_ident[:])
                nc.vector.tensor_scalar(wH[:, a * P:(a + 1) * P], tp2[:],
                                        WL_STD, None, ALU.mult)
            nc.vector.tensor_scalar(cH[:], cH[:], C_MEAN, None, ALU.add)
            nc.vector.tensor_scalar(wH[:], wH[:], WL_MEAN, None, ALU.add)

            # nearest conc index (times 4): rc4 = 4*clip(round(c/4), 0, 15)
            r1 = hpool.tile([P, FC], F32, tag="r1")
            nc.vector.tensor_scalar(r1[:], cH[:], 0.25, MAGIC, ALU.mult, ALU.add)
            rc = hpool.tile([P, FC], F32, tag="rc")
            nc.vector.tensor_scalar(rc[:], r1[:], MAGIC, 0.0, ALU.subtract, ALU.max)
            rc4 = hpool.tile([P, FC], F32, tag="rc4")
            nc.vector.tensor_scalar(rc4[:], rc[:], 15.0, 4.0, ALU.min, ALU.mult)
            # delta_c = cH - rc4  (single rounded subtract, matches reference)
            dC = hpool.tile([P, FC], F32, tag="dC")
            nc.vector.scalar_tensor_tensor(dC[:], rc4[:], -1.0, cH[:],
                                           ALU.mult, ALU.add)
            # chit = (dC < 0.1) & (dC > -0.1)
            uC = hpool.tile([P, FC], F32, tag="uC")
            nc.vector.tensor_scalar(uC[:], dC[:], 0.1, None, ALU.is_lt)
            chit = hpool.tile([P, FC], F32, tag="chit")
            nc.vector.scalar_tensor_tensor(chit[:], dC[:], -0.1, uC[:],
                                           ALU.is_gt, ALU.mult)

            # nearest wavelength index: rw6 = clip(round(wl), 200, 800) - 200
            r1w = hpool.tile([P, FC], F32, tag="r1w")
            nc.vector.tensor_scalar(r1w[:], wH[:], MAGIC, None, ALU.add)
            rw = hpool.tile([P, FC], F32, tag="rw")
            nc.vector.tensor_scalar(rw[:], r1w[:], MAGIC, 200.0,
                                    ALU.subtract, ALU.max)
            rw6 = hpool.tile([P, FC], F32, tag="rw6")
            nc.vector.tensor_scalar(rw6[:], rw[:], 800.0, 200.0,
                                    ALU.min, ALU.subtract)
            # ndW = (rw6 + 200) - wH  == -delta_w (sign irrelevant for the test)
            ndW = hpool.tile([P, FC], F32, tag="ndW")
            nc.vector.scalar_tensor_tensor(ndW[:], rw6[:], 200.0, wH[:],
                                           ALU.add, ALU.subtract)
            uW = hpool.tile([P, FC], F32, tag="uW")
            nc.vector.tensor_scalar(uW[:], ndW[:], 0.1, None, ALU.is_lt)
            whit = hpool.tile([P, FC], F32, tag="whit")
            nc.vector.scalar_tensor_tensor(whit[:], ndW[:], -0.1, uW[:],
                                           ALU.is_gt, ALU.mult)

            mask = hpool.tile([P, FC], mybir.dt.uint8, tag="mask")
            nc.vector.tensor_tensor(mask[:], chit[:], whit[:], ALU.mult)

            # flat gather index g = c_idx*601 + wl_idx = rc4*150.25 + rw6
            gf = hpool.tile([P, FC], F32, tag="gf")
            nc.vector.scalar_tensor_tensor(gf[:], rc4[:], 150.25, rw6[:],
                                           ALU.mult, ALU.add)
            g16 = hpool.tile([P, FC], I16, tag="g16")
            nc.vector.tensor_copy(g16[:], gf[:])

            # gpsimd flat-table gather: per 16-partition group, 4096 idxs
            dst = hpool.tile([P, BC // 8], F32, tag="dst")
            nc.gpsimd.ap_gather(dst[:], s_tbl[:], g16[:], channels=P,
                                num_elems=TBL, d=1, num_idxs=BC // 8)
            # un-wrap the group-sequence-major gather output to T-major per
            # partition: per-partition local_scatter of the two u16 lanes
            exactT = hpool.tile([P, FC], F32, tag="exactT")
            nc.gpsimd.local_scatter(
                exactT[:].bitcast(mybir.dt.uint16),
                dst[:].bitcast(mybir.dt.uint16),
                s_sidx[:],
                channels=P, num_elems=2 * FC, num_idxs=2 * (BC // 8))

            # =========================================================
            # MLP (feature-major, bf16 matmuls)
            # =========================================================
            ps4 = pl4.tile([P, FC], F32, tag="l4")  # persistent interp psum
            for u in range(N_U):
                b, t = u // 32, u % 32
                ps1 = pl1.tile([P, UW], F32, tag="l1")
                nc.tensor.matmul(ps1[:], s_w1[64 * b:64 * b + 64,
                                              128 * t:128 * (t + 1)],
                                 s_xin[64 * b:64 * b + 64, :])
                h1 = mpool.tile([P, UW], BF16, tag="h1")
                nc.scalar.activation(h1[:], ps1[:], ACTF.Tanh, bias=s_b1[:])

                ps2a = pl2a.tile([P, UW], F32, tag="l2a")
                nc.tensor.matmul(ps2a[:], s_w2[:, 0:128], h1[:])
                h2a = mpool.tile([P, UW], BF16, tag="h2a")
                nc.scalar.activation(h2a[:], ps2a[:], ACTF.Tanh, bias=s_b2a[:])

                ps2b = pl2b.tile([P, UW], F32, tag="l2b")
                nc.tensor.matmul(ps2b[:], s_w2[:, 128:256], h1[:])
                h2b = mpool.tile([P, UW], BF16, tag="h2b")
                nc.scalar.activation(h2b[:], ps2b[:], ACTF.Tanh, bias=s_b2b[:])

                ps3 = pl3.tile([P, UW], F32, tag="l3")
                nc.tensor.matmul(ps3[:], s_w3[:, 0:128], h2a[:],
                                 start=True, stop=False)
                nc.tensor.matmul(ps3[:], s_w3[:, 128:256], h2b[:],
                                 start=False, stop=True)
                h3 = mpool.tile([P, UW], BF16, tag="h3")
                nc.scalar.activation(h3[:], ps3[:], ACTF.Tanh, bias=s_b3[:])

                # L4: batch-on-partition columns of the persistent psum tile
                for v in range(4):
                    T = 4 * u + v
                    nc.tensor.matmul(ps4[:, T:T + 1],
                                     h3[:, 128 * v:128 * (v + 1)], s_w4[:])

            # =========================================================
            # Merge: out_sb = interp; overwrite exact hits; back to natural
            # =========================================================
            out_sb = hpool.tile([P, FC], F32, tag="out_sb")
            nc.vector.tensor_scalar(out_sb[:], ps4[:], s_b4[:], None, ALU.add)
            if debug_outputs:
                nc.sync.dma_start(d_mask, mask[:])
                nc.sync.dma_start(d_gf, gf[:])
                nc.sync.dma_start(d_ex, exactT[:])
                nc.sync.dma_start(d_it, out_sb[:])
                nc.sync.dma_start(d_dst, dst[:])

            # mask lives in H-layout (free u' = 128*a + q); out_sb/exactT free
            # is T = 2*q + a.  Iterate T-order via rearranged APs.
            nc.vector.copy_predicated(
                out_sb[:].rearrange("p (q a) -> p q a", a=2),
                mask[:].rearrange("p (a q) -> p q a", a=2),
                exactT[:].rearrange("p (a q) -> p q a", a=2),
            )

            # natural layout: nat_a[p, i] = out_sb[i, 2p + a]
            onat = hpool.tile([P, FC], F32, tag="onat")
            for a in range(2):
                tp = ptr.tile([P, P], F32, tag="tr")
                nc.tensor.transpose(tp[:], out_sb[:, a::2], s_ident[:])
                nc.vector.tensor_copy(onat[:, a * P:(a + 1) * P], tp[:])
            nc.sync.dma_start(d_out, onat[:])

    nc.finalize()
    return nc


_NC_CACHE = {}


def _get_nc():
    if "nc" not in _NC_CACHE:
        _NC_CACHE["nc"] = build_nc()
    return _NC_CACHE["nc"]


def make_in_maps(c_norm, wl_norm, train_concs, train_wavelengths, abs_matrix,
                 w1, b1, w2, b2, w3, b3, w4, b4):
    bf16 = ml_dtypes.bfloat16
    f32 = np.float32

    A = np.asarray(abs_matrix, f32)
    m = A.mean(dtype=f32).astype(f32)
    s = A.std(dtype=f32).astype(f32)
    tbl_row = ((A.ravel() - m) / s).astype(f32)
    tbl = np.ascontiguousarray(np.broadcast_to(tbl_row, (P, TBL)))

    w1 = np.asarray(w1, f32)
    w1all = np.zeros((P, 4096), bf16)
    for u in range(N_U):
        b_, t_ = u // 32, u % 32
        w1all[64 * b_ + 2 * t_, 128 * t_:128 * (t_ + 1)] = w1[0].astype(bf16)
        w1all[64 * b_ + 2 * t_ + 1, 128 * t_:128 * (t_ + 1)] = w1[1].astype(bf16)

    w2sb = np.asarray(w2, f32).astype(bf16)                      # [128, 256]
    w3 = np.asarray(w3, f32)
    w3ab = np.concatenate([w3[0:128, :], w3[128:256, :]], axis=1).astype(bf16)
    w4sb = np.asarray(w4, f32).astype(bf16)                      # [128, 1]

    b1r = np.asarray(b1, f32).reshape(P, 1)
    b2 = np.asarray(b2, f32)
    b2a = b2[0:128].reshape(P, 1)
    b2b = b2[128:256].reshape(P, 1)
    b3r = np.asarray(b3, f32).reshape(P, 1)
    b4r = np.full((P, 1), np.asarray(b4, f32).ravel()[0], f32)
    ident = np.eye(P, dtype=f32)

    # local_scatter indices: partition i keeps gather-sequence slots
    # j = 16*T + (i%16) and routes fp32 u16-lane 2j+l -> out slot 2T+l.
    nj = BC // 8                                   # 4096 per group
    j = np.arange(nj)
    sidx = np.full((P, 2 * nj), -1, np.int16)
    for r in range(16):
        sel = (j % 16) == r
        T = (j[sel] // 16).astype(np.int64)
        rows = slice(0, P)
        for l_ in range(2):
            col = 2 * j[sel] + l_
            val = (2 * T + l_).astype(np.int16)
            for i in range(r, P, 16):
                sidx[i, col] = val

    c_norm = np.asarray(c_norm, f32)
    wl_norm = np.asarray(wl_norm, f32)

    in_maps = []
    for i in range(N_CORES):
        sl = slice(i * BC, (i + 1) * BC)
        cs, ws = c_norm[sl], wl_norm[sl]
        xin = np.empty((P, UW), bf16)
        xin[0::2] = cs.reshape(N_U, UW).astype(bf16)
        xin[1::2] = ws.reshape(N_U, UW).astype(bf16)
        in_maps.append({
            "cn": cs.reshape(P, FC),
            "wn": ws.reshape(P, FC),
            "xin": xin,
            "w1all": w1all, "w2": w2sb, "w3ab": w3ab, "w4": w4sb,
            "b1": b1r, "b2a": b2a, "b2b": b2b, "b3": b3r, "b4r": b4r,
            "tbl": tbl, "ident": ident, "sidx": sidx,
        })
    return in_maps


def kernel(**inputs):
    nc = _get_nc()
    in_maps = make_in_maps(**inputs)
    res = run_bass_kernel_spmd(nc, in_maps, core_ids=list(range(N_CORES)))
    parts = [res.results[i]["out"].reshape(BC) for i in range(N_CORES)]
    return np.concatenate(parts).astype(np.float32)
